# revision 3
# baseline (speedup 1.0000x reference)
"""Trainium2 Bass kernel for nn_ConjunctionLayer (fuzzy-logic AND layer).

out[b, n] = prod_d (1 - (1 - x[b,d]) * W[n,d])

Reformulation: with u = 1-x (in [0,1]) and w = W (in [0, 0.1)), z = u*w in
[0, 0.1), so

    log out[b,n] = sum_d log(1 - z_bdn)  ~=  -sum_{k=1..K} a_k * sum_d u^k w^k

where the inner sum over d is a matmul of elementwise powers.  a_k are
least-squares Chebyshev-node coefficients of -log(1-z)/z on [0, 0.1]
(K=3: per-element approx error < 1e-6, far below fp32 reference noise).

    out = exp(-(a_1 * u@w.T + a_2 * u^2@(w^2).T + a_3 * u^3@(w^3).T))

The k=1 term dominates (|S1| ~ 13) and is computed with a bf16 hi/lo split
(3 matmul passes: uh*wh + uh*wl + ul*wh); k>=2 terms use plain bf16 power
chains (their magnitude is ~0.3 / ~0.01 so bf16 noise there is ~1e-4 of out).

Sharding: data-parallel over batch. 8 cores x 128 batch rows each; W is
replicated. Each core runs the identical program on its x shard.
"""

import numpy as np

import concourse.bacc as bacc
import concourse.bass as bass
import concourse.mybir as mybir
import concourse.tile as tile
from concourse.alu_op_type import AluOpType
from concourse.bass_utils import run_bass_kernel_spmd
from concourse.masks import make_identity

B, D, N = 1024, 512, 512
NCORES = 8
BS = B // NCORES          # batch rows per core
KC = D // 128             # contraction chunks of 128

# Degree-3 fit of -log(1-z)/z on [0, 0.1] (see numerics_check.py)
A1 = 1.00000904
A2 = 0.49839935
A3 = 0.37467614

FP32 = mybir.dt.float32
BF16 = mybir.dt.bfloat16


def _emit(ctx, tc, nc, x_d, w_d, o_d):
    pool = ctx.enter_context(tc.tile_pool(name="sbuf", bufs=1))
    psum = ctx.enter_context(tc.tile_pool(name="psum", bufs=1, space="PSUM"))

    ident = pool.tile([128, 128], FP32)
    make_identity(nc, ident)

    # ---- loads ----
    xs = pool.tile([128, D], FP32)
    nc.sync.dma_start(xs, x_d)
    w_nat = pool.tile([128, KC, D], FP32)   # w_nat[p, t, d] = W[t*128+p, d]
    for t in range(KC):
        nc.sync.dma_start(w_nat[:, t, :], w_d[t * 128:(t + 1) * 128, :])

    # ---- PE transposes (fp32) ----
    # xT[p, kc, b] = x[b, kc*128+p]
    ps_x = psum.tile([128, D], FP32)
    for kc in range(KC):
        nc.tensor.transpose(ps_x[:, kc * 128:(kc + 1) * 128],
                            xs[:, kc * 128:(kc + 1) * 128], ident)
    xT = pool.tile([128, KC, BS], FP32)
    nc.vector.tensor_copy(xT, ps_x)

    # wT[p, kc, n] = W[n, kc*128+p]
    wT = pool.tile([128, KC, N], FP32)
    for kc in range(KC):
        ps_w = psum.tile([128, N], FP32, name=f"ps_w{kc}")
        for t in range(KC):
            nc.tensor.transpose(ps_w[:, t * 128:(t + 1) * 128],
                                w_nat[:, t, kc * 128:(kc + 1) * 128], ident)
        nc.vector.tensor_copy(wT[:, kc, :], ps_w)

    # ---- u-side elementwise ([128, 512] each) ----
    uT = pool.tile([128, KC, BS], FP32)     # u = 1 - x
    nc.vector.tensor_scalar(uT, xT, -1.0, 1.0, AluOpType.mult, AluOpType.add)
    t1 = pool.tile([128, KC, BS], FP32)     # a1 * u = -a1*x + a1
    nc.vector.tensor_scalar(t1, xT, -A1, A1, AluOpType.mult, AluOpType.add)
    uh = pool.tile([128, KC, BS], BF16)
    nc.scalar.copy(uh, t1)
    ul = pool.tile([128, KC, BS], BF16)     # t1 - uh
    nc.vector.scalar_tensor_tensor(ul, uh, -1.0, t1, AluOpType.mult, AluOpType.add)
    ub = pool.tile([128, KC, BS], BF16)
    nc.scalar.copy(ub, uT)
    u2b = pool.tile([128, KC, BS], BF16)    # u^2
    nc.vector.tensor_mul(u2b, ub, ub)
    u2s = pool.tile([128, KC, BS], BF16)    # a2 * u^2
    nc.vector.tensor_scalar_mul(u2s, u2b, A2)
    u3s = pool.tile([128, KC, BS], BF16)    # (u^2 * a3) * u
    nc.vector.scalar_tensor_tensor(u3s, u2b, A3, ub, AluOpType.mult, AluOpType.mult)

    # ---- w-side elementwise ([128, 2048] each) ----
    wh = pool.tile([128, KC, N], BF16)
    nc.vector.tensor_copy(wh, wT)
    wl = pool.tile([128, KC, N], BF16)      # wT - wh
    nc.vector.scalar_tensor_tensor(wl, wh, -1.0, wT, AluOpType.mult, AluOpType.add)
    w2 = pool.tile([128, KC, N], BF16)
    nc.gpsimd.tensor_mul(w2, wh, wh)
    w3 = pool.tile([128, KC, N], BF16)
    nc.gpsimd.tensor_mul(w3, w2, wh)

    # ---- matmul accumulation: S[b, n] in one PSUM bank ----
    ps_out = psum.tile([128, N], FP32, name="ps_out")
    passes = [(uh, wh), (uh, wl), (ul, wh), (u2s, w2), (u3s, w3)]
    n_mm = len(passes) * KC
    i = 0
    for (ut, wt) in passes:
        for kc in range(KC):
            nc.tensor.matmul(ps_out, ut[:, kc, :], wt[:, kc, :],
                             start=(i == 0), stop=(i == n_mm - 1))
            i += 1

    # ---- out = exp(-S) ----
    outs = pool.tile([128, N], FP32)
    nc.scalar.activation(outs, ps_out, mybir.ActivationFunctionType.Exp,
                         scale=-1.0)
    nc.sync.dma_start(o_d, outs)


_CACHE = {}


def _build():
    if "nc" in _CACHE:
        return _CACHE["nc"]
    nc = bacc.Bacc("TRN2", target_bir_lowering=False, debug=False,
                   num_devices=NCORES)
    x_d = nc.dram_tensor("x", [BS, D], FP32, kind="ExternalInput").ap()
    w_d = nc.dram_tensor("W", [N, D], FP32, kind="ExternalInput").ap()
    o_d = nc.dram_tensor("out", [BS, N], FP32, kind="ExternalOutput").ap()
    from contextlib import ExitStack
    with tile.TileContext(nc) as tc, ExitStack() as ctx:
        _emit(ctx, tc, nc, x_d, w_d, o_d)
    nc.compile()
    _CACHE["nc"] = nc
    return nc


def kernel(x: np.ndarray, W: np.ndarray) -> np.ndarray:
    nc = _build()
    x = np.ascontiguousarray(np.asarray(x, np.float32))
    W = np.ascontiguousarray(np.asarray(W, np.float32))
    in_maps = [{"x": x[i * BS:(i + 1) * BS], "W": W} for i in range(NCORES)]
    res = run_bass_kernel_spmd(nc, in_maps, list(range(NCORES)))
    return np.concatenate([res.results[i]["out"] for i in range(NCORES)], axis=0)
